# revision 52
# baseline (speedup 1.0000x reference)
"""Gemma3 sliding-window attention on 8 Trainium2 NeuronCores.

Sharding: core c handles batch b=c//4 and head-group g=c%4 (4 of 16 q heads,
2 of 8 kv heads). wq/wk/wv column-split, wo row-split; the 4 partial outputs
per batch are summed on host (no device collectives).

v2: all matmul operands in bf16 (fp32 PSUM accumulation), single-instruction
batched DMA loads from host-prepacked layouts, per-q-tile(128) attention with
the softmax denominator computed as a 129th V column in a [q,d]-oriented PV
matmul, XBAR DMA transposes to return attn to [d,q] for the output
projection, and software-pipelined instruction issue so the PE never waits
on the RMSNorm/RoPE vector chains.
"""

import math
import numpy as np
import ml_dtypes

import concourse.bacc as bacc
import concourse.mybir as mybir
import concourse.tile as tile
from concourse.bass_utils import run_bass_kernel_spmd

dt = mybir.dt
AFT = mybir.ActivationFunctionType
BF = dt.bfloat16
F32 = dt.float32

B, S, H = 2, 2048, 2048
NQC, NKVC, D = 4, 2, 128          # per-core heads
WIN = 1024
EPS = 1e-6
THETA = 10000.0
P = 128
SCP = 512                          # phase-1 seq chunk
NCH = S // SCP                     # 4
NHT = H // P                       # 16
NST = S // P                       # 16
WT = WIN // P                      # 8 (window in tiles)
LAG = 2                            # attention PV pipeline depth (pair units)

_CACHE = {}


def _groups_for(t0, u0):
    """k-tile groups for one q tile: runs of <=4 tiles, diagonal tile alone
    last (so its [128,128] exp/mask stays separate)."""
    ts = list(range(t0, u0 + 1))
    if len(ts) == 1:
        return [ts]
    body, diag = ts[:-1], ts[-1:]
    gs = [body[i:i + 4] for i in range(0, len(body), 4)]
    gs.append(diag)
    return gs


def _build_nc():
    if "nc" in _CACHE:
        return _CACHE["nc"]
    nc = bacc.Bacc("TRN2", target_bir_lowering=False, debug=False, num_devices=8)

    F8 = dt.float8e4
    DR = mybir.MatmulPerfMode.DoubleRow
    # hi/lo fp8 pairs: x ~= hi + lo to ~0.1% rms; DoubleRow matmuls run the
    # (hi,hi), (hi,lo), (lo,hi) cross terms at 0.5 cyc/row over ht-pairs.
    hs_d = nc.dram_tensor("hs", [P, NCH, 2, NHT // 2, 2, 4, P], F8,
                          kind="ExternalInput").ap()
    wq_d = nc.dram_tensor("wq", [P, 2, NHT // 2, 2, NQC * D], F8,
                          kind="ExternalInput").ap()
    wk_d = nc.dram_tensor("wk", [P, 2, NHT // 2, 2, NKVC * D], F8,
                          kind="ExternalInput").ap()
    wv_d = nc.dram_tensor("wv", [P, 2, NHT // 2, 2, NKVC * D], F8,
                          kind="ExternalInput").ap()
    wo_d = nc.dram_tensor("wo", [P, NQC, H], BF, kind="ExternalInput").ap()
    tabs_d = nc.dram_tensor("tabs", [P, 4, S], BF, kind="ExternalInput").ap()
    rots_d = nc.dram_tensor("rots", [P, 2, P], BF, kind="ExternalInput").ap()
    msk_d = nc.dram_tensor("msk", [P, 2, 2, P], BF, kind="ExternalInput").ap()
    y_d = nc.dram_tensor("y", [P, NHT, S], BF, kind="ExternalOutput").ap()

    with nc.allow_low_precision(reason="bf16 kernel; rel-err budget 2e-2"), \
         tile.TileContext(nc) as tc:
        with (
            tc.tile_pool(name="const", bufs=1) as cpool,
            tc.tile_pool(name="qkv", bufs=1) as qkv,
            tc.tile_pool(name="wts", bufs=1) as wts,
        ):
            msk_sb = cpool.tile([P, 2, 2, P], BF, tag="msk")
            rots_sb = cpool.tile([P, 2, P], BF, tag="rots")
            ones_sb = cpool.tile([P, P], BF, tag="ones")
            eps_sb = cpool.tile([P, 1], F32, tag="eps")
            nc.vector.memset(ones_sb[:], 1.0)
            nc.vector.memset(eps_sb[:], EPS)
            dm_sb = msk_sb[:, 0, :, :]
            em_sb = msk_sb[:, 1, :, :]

            # weight loads: wv first (v-projection is the startup filler work),
            # then wk (k heads run before q heads), wq, wo last-needed.
            wv_sb = wts.tile([P, 2, NHT // 2, 2, NKVC * D], F8, tag="wv")
            wk_sb = wts.tile([P, 2, NHT // 2, 2, NKVC * D], F8, tag="wk")
            wq_sb = wts.tile([P, 2, NHT // 2, 2, NQC * D], F8, tag="wq")
            wo_sb = wts.tile([P, NQC, H], BF, tag="wo")

            qn_sb = qkv.tile([P, NQC, S], BF, tag="qn")
            kn_sb = qkv.tile([P, NKVC, S], BF, tag="kn")
            v_sb = qkv.tile([P, NST, NKVC, D + 1], BF, tag="v")
            nc.vector.memset(v_sb[:, :, :, D:D + 1], 64.0)

            # ---------------- phase 1: QKV projections + RMSNorm + RoPE ----
            # per (chunk, head): PE proj chain -> Act copy -> DVE square /
            # rope muls; the sum-of-squares and rotation matmuls for head m
            # are issued after head m+1's projection chain so PE never waits.
            with (
                tc.tile_pool(name="hsp", bufs=2) as hsp,
                tc.tile_pool(name="tabp", bufs=2) as tabp,
                tc.tile_pool(name="cpp", bufs=4) as cpp,
                tc.tile_pool(name="t1", bufs=3) as t1p,
                tc.tile_pool(name="t2", bufs=3) as t2p,
                tc.tile_pool(name="t3", bufs=3) as t3p,
                tc.tile_pool(name="t4", bufs=3) as t4p,
                tc.tile_pool(name="t5", bufs=4) as t5p,
                tc.tile_pool(name="t6", bufs=2) as t6p,
                tc.tile_pool(name="pp", bufs=2, space="PSUM") as ppp,
                tc.tile_pool(name="prb", bufs=2, space="PSUM") as prbp,
                tc.tile_pool(name="pvb", bufs=2, space="PSUM") as pvbp,
                tc.tile_pool(name="psv", bufs=2, space="PSUM") as psvp,
            ):
                # heads order: k0, k1, q0..q3 (wk arrives before wq)
                HEADS = [("k", 0), ("k", 1), ("q", 0), ("q", 1), ("q", 2), ("q", 3)]
                pend = []  # deferred norm/rope finishes (2-deep pipeline)

                def proj_chain(out_ps, w_sb8, mcols, hs_t):
                    first = True
                    for wi, xi in ((0, 0), (0, 1), (1, 0)):
                        for tp in range(NHT // 2):
                            nc.tensor.matmul(
                                out_ps[:], w_sb8[:, wi, tp, :, mcols],
                                hs_t[:, xi, tp, :, :, :],
                                perf_mode=DR, start=first,
                                stop=(wi == 1 and tp == NHT // 2 - 1))
                            first = False

                def v_chain(out_ps, hs_t, ss):
                    first = True
                    for wi, xi in ((0, 0), (0, 1), (1, 0)):
                        for tp in range(NHT // 2):
                            nc.tensor.matmul(
                                out_ps[:], hs_t[:, xi, tp, :, ss, :],
                                wv_sb[:, wi, tp, :, :],
                                perf_mode=DR, start=first,
                                stop=(wi == 1 and tp == NHT // 2 - 1))
                            first = False

                def finish(st):
                    kind, m, pp, cp, u_t, s0, tab_t = st
                    sq = t1p.tile([P, SCP], BF, tag="sq")
                    nc.vector.tensor_mul(sq[:], cp[:], cp[:])
                    rb = prbp.tile([P, SCP], F32, tag="rb")
                    rot = rots_sb[:, 0, :] if kind == "q" else rots_sb[:, 1, :]
                    nc.tensor.matmul(rb[:], rot, cp[:], start=True, stop=True)
                    vb = pvbp.tile([P, SCP], F32, tag="vb")
                    nc.tensor.matmul(vb[:], ones_sb[:], sq[:], start=True, stop=True)
                    sd = t2p.tile([P, SCP], F32, tag="sd")
                    nc.scalar.activation(sd[:], vb[:], AFT.Sqrt, bias=eps_sb[:],
                                         scale=1.0 / D)
                    inv = t3p.tile([P, SCP], BF, tag="inv")
                    nc.vector.reciprocal(inv[:], sd[:])
                    # tsin: rb (PSUM) is ready late
                    tsin = t4p.tile([P, SCP], BF, tag="tsin")
                    sin_t = tab_t[:, 1 if kind == "q" else 3, :]
                    nc.vector.tensor_mul(tsin[:], rb[:], sin_t)
                    nc.vector.tensor_add(u_t[:], u_t[:], tsin[:])
                    dst = qn_sb if kind == "q" else kn_sb
                    nc.vector.tensor_mul(dst[:, m, s0:s0 + SCP], u_t[:], inv[:])

                for sc in range(NCH):
                    s0 = sc * SCP
                    hs_sb = hsp.tile([P, 2, NHT // 2, 2, 4, P], F8, tag="hs")
                    if sc == 0:
                        # startup-critical order: hi parts first (the hi-hi
                        # chain leads each accumulation), v before k/q.
                        nc.sync.dma_start(out=wv_sb[:, 0], in_=wv_d[:, 0])
                        nc.sync.dma_start(out=hs_sb[:, 0, 0:4], in_=hs_d[:, 0, 0, 0:4])
                        nc.sync.dma_start(out=hs_sb[:, 0, 4:8], in_=hs_d[:, 0, 0, 4:8])
                        nc.sync.dma_start(out=wv_sb[:, 1], in_=wv_d[:, 1])
                        nc.sync.dma_start(out=hs_sb[:, 1], in_=hs_d[:, 0, 1])
                        nc.sync.dma_start(out=wk_sb[:], in_=wk_d[:])
                        nc.sync.dma_start(out=msk_sb[:], in_=msk_d[:])
                        nc.sync.dma_start(out=rots_sb[:], in_=rots_d[:])
                        nc.sync.dma_start(out=wq_sb[:], in_=wq_d[:])
                    else:
                        nc.sync.dma_start(out=hs_sb[:], in_=hs_d[:, sc])
                    tab_sb = tabp.tile([P, 4, SCP], BF, tag="tab")
                    nc.sync.dma_start(out=tab_sb[:], in_=tabs_d[:, :, s0:s0 + SCP])
                    if sc == 0:
                        nc.sync.dma_start(out=wo_sb[:], in_=wo_d[:])

                    # v projection: natural [seq, d] layout + filler work
                    for ss in range(SCP // P):
                        st_g = sc * (SCP // P) + ss
                        pv = psvp.tile([P, NKVC * D], F32, tag="pv")
                        v_chain(pv, hs_sb, ss)
                        nc.vector.tensor_copy(v_sb[:, st_g, :, 0:D], pv[:])
                        if ss == 0 and pend:
                            # finish the previous chunk's last head here: its
                            # DVE ops land ahead of the remaining v copies
                            finish(pend.pop(0))

                    for kind, m in HEADS:
                        w_sb = wq_sb if kind == "q" else wk_sb
                        pp = ppp.tile([P, SCP], F32, tag="pp")
                        proj_chain(pp, w_sb, slice(m * D, (m + 1) * D), hs_sb)
                        cp = cpp.tile([P, SCP], BF, tag="cp")
                        nc.scalar.copy(cp[:], pp[:])
                        u_t = t5p.tile([P, SCP], BF, tag="u")
                        cos_t = tab_sb[:, 0 if kind == "q" else 2, :]
                        nc.vector.tensor_mul(u_t[:], cp[:], cos_t)
                        pend.append((kind, m, pp, cp, u_t, s0, tab_sb))
                        if len(pend) > 1:
                            finish(pend.pop(0))
                for st in pend:
                    finish(st)

            # ---------------- phase 2+3: attention + output projection -----
            with (
                tc.tile_pool(name="pb", bufs=LAG + 2) as pbp,
                tc.tile_pool(name="invp", bufs=4) as invp,
                tc.tile_pool(name="aq", bufs=2) as aqp,
                tc.tile_pool(name="aT", bufs=2) as aTp,
                tc.tile_pool(name="ysb", bufs=2) as ysp,
                tc.tile_pool(name="psc", bufs=4, space="PSUM") as pscp,
                tc.tile_pool(name="pa", bufs=2, space="PSUM") as pap,
                tc.tile_pool(name="psy", bufs=2, space="PSUM") as psyp,
            ):
                queue = []
                slab_tiles = {}

                def emit_scores(kvh, u0):
                    # paired unit: both q heads of this kv head at once
                    h0 = 2 * kvh
                    t0 = max(0, u0 - WT)
                    n = u0 - t0 + 1
                    p_t = pbp.tile([P, WT + 1, 2, P], BF, tag="p", name="p_t")
                    qn_sl = qn_sb[:, h0:h0 + 2, u0 * P:(u0 + 1) * P]
                    for g0 in range(0, n, 2):
                        gn = min(2, n - g0)
                        sc_t = pscp.tile([P, 2, 2, P], F32, tag="sc",
                                         name="sc_t")
                        for i in range(gn):
                            t = t0 + g0 + i
                            nc.tensor.matmul(
                                sc_t[:, i, :, :],
                                kn_sb[:, kvh, t * P:(t + 1) * P],
                                qn_sl, start=True, stop=True)
                        nc.scalar.activation(p_t[:, g0:g0 + gn, :, :],
                                             sc_t[:, 0:gn, :, :], AFT.Exp)
                        for i in range(gn):
                            t = t0 + g0 + i
                            if t == u0:
                                blk = p_t[:, g0 + i, :, :]
                                nc.vector.tensor_mul(blk, blk, dm_sb)
                            elif u0 >= WT and t == u0 - WT:
                                blk = p_t[:, g0 + i, :, :]
                                nc.vector.tensor_mul(blk, blk, em_sb)
                    return (kvh, u0, t0, p_t)

                def emit_pv(st):
                    kvh, u0, t0, p_t = st
                    n = u0 - t0 + 1
                    slab = slab_tiles[u0 // 4]
                    for hh in range(2):
                        h = 2 * kvh + hh
                        a_t = pap.tile([P, D + 1], F32, tag="a")
                        for i in range(n):
                            nc.tensor.matmul(
                                a_t[:], p_t[:, i, hh, :],
                                v_sb[:, t0 + i, kvh, :],
                                start=(i == 0), stop=(i == n - 1))
                        inv = invp.tile([P, 1], F32, tag="inv")
                        nc.vector.reciprocal(inv[:], a_t[:, D:D + 1])
                        nc.vector.tensor_scalar_mul(slab[:, h, u0 % 4, :],
                                                    a_t[:, 0:D], inv[:])

                def emit_transpose(s4, h):
                    # Act HWDGE queue: avoids head-of-line blocking behind the
                    # SP queue's y-out DMAs (which wait on DVE copies). Issued
                    # one per unit so the Act SEQ time (~0.7us per DMA) does
                    # not delay exp dispatch in a lump.
                    if h == 0:
                        aT = aTp.tile([P, NQC, 4, P], BF, tag="aT", name="aT")
                        slab_tiles[("T", s4)] = aT
                    aT = slab_tiles[("T", s4)]
                    slab = slab_tiles[s4]
                    nc.sync.dma_start_transpose(out=aT[:, h, :, :],
                                                  in_=slab[:, h, :, :])

                op_queue = []  # (s4, mo) outproj chains, spread across units
                op_state = {}

                def emit_outproj_chain():
                    if not op_queue:
                        return
                    s4, mo = op_queue.pop(0)
                    aT = slab_tiles[("T", s4)]
                    mog, mo4 = divmod(mo, 4)
                    if mo4 == 0:
                        op_state["y"] = ysp.tile([P, 4, SCP], BF, tag="y",
                                                 name="y_t")
                    y_t = op_state["y"]
                    yp = psyp.tile([P, SCP], F32, tag="yp")
                    for h in range(NQC):
                        nc.tensor.matmul(
                            yp[:], wo_sb[:, h, mo * P:(mo + 1) * P],
                            aT[:, h, :, :],
                            start=(h == 0), stop=(h == NQC - 1))
                    nc.vector.tensor_copy(y_t[:, mo4, :], yp[:])
                    if mo4 == 3:
                        nc.sync.dma_start(
                            out=y_d[:, mog * 4:(mog + 1) * 4,
                                    s4 * SCP:(s4 + 1) * SCP],
                            in_=y_t[:])

                def emit_outproj(s4):
                    op_queue.extend((s4, mo) for mo in range(NHT))

                # descending u0: the big steady-state units come first and
                # prime the PV pipeline; the small ramp units land at the end
                # where the outproj slabs provide PE filler work.
                for u0 in range(NST - 1, -1, -1):
                    if u0 % 4 == 3:
                        slab_tiles[u0 // 4] = aqp.tile([P, NQC, 4, P], BF,
                                                       tag="aq", name="aq")
                    if u0 % 4 == 0 and u0 <= NST - 8:
                        emit_outproj(u0 // 4 + 1)
                    for kvh in range(NKVC):
                        queue.append(emit_scores(kvh, u0))
                        if len(queue) > LAG:
                            emit_pv(queue.pop(0))
                        if u0 == 0 and queue:
                            emit_pv(queue.pop(0))  # drain early for the tail
                        if u0 % 4 == 2 and u0 <= NST - 6:
                            emit_transpose(u0 // 4 + 1, 2 * kvh)
                            emit_transpose(u0 // 4 + 1, 2 * kvh + 1)
                        emit_outproj_chain()
                        emit_outproj_chain()
                while queue:
                    emit_pv(queue.pop(0))
                    emit_outproj_chain()
                    emit_outproj_chain()
                # first slab (last processed): split transposes across both
                # HWDGE queues to halve the serial latency in the tail
                aT = aTp.tile([P, NQC, 4, P], BF, tag="aT", name="aT")
                slab_tiles[("T", 0)] = aT
                slab = slab_tiles[0]
                for h in range(NQC):
                    eng = nc.scalar if h % 2 == 0 else nc.sync
                    eng.dma_start_transpose(out=aT[:, h, :, :],
                                            in_=slab[:, h, :, :])
                emit_outproj(0)
                while op_queue:
                    emit_outproj_chain()

    nc.compile()
    _CACHE["nc"] = nc
    return nc


def _host_inputs(hidden_states, wq, wk, wv, wo, q_norm_weight, k_norm_weight):
    """Per-core input dicts (8 cores: c = 4*b + g)."""
    bf = ml_dtypes.bfloat16
    f = np.float32
    scale = 1.0 / math.sqrt(D)
    inv_freq = 1.0 / (THETA ** (np.arange(0, D, 2, dtype=np.float64) / D))
    t = np.arange(S, dtype=np.float64)
    freqs = np.outer(t, inv_freq)
    emb = np.concatenate([freqs, freqs], axis=-1)          # [S, D]
    cosT = np.cos(emb).T.astype(np.float64)                # [D, S]
    sinT = np.sin(emb).T.astype(np.float64)
    qw = (1.0 + q_norm_weight).astype(np.float64)
    kw = (1.0 + k_norm_weight).astype(np.float64)

    # rotate-half matrices with norm weights folded (lhsT layout, like
    # baseline): rb = rots.T @ x = (R * w) @ x
    R = np.zeros((D, D), np.float64)
    hh = D // 2
    for i in range(hh):
        R[i, i + hh] = -1.0
        R[i + hh, i] = 1.0
    rqT = np.ascontiguousarray((R * qw[None, :]).T)
    rkT = np.ascontiguousarray((R * kw[None, :]).T)
    rots = np.stack([rqT, rkT], axis=1).astype(bf)         # [D, 2, D]

    tabs = np.stack([
        cosT * qw[:, None] * scale,
        sinT * scale,
        cosT * kw[:, None],
        sinT,
    ], axis=1).astype(bf)                                  # [D, 4, S]

    r = np.arange(P)[:, None]
    c = np.arange(P)[None, :]
    dmask = np.where(c >= r, 1.0, 0.0)                     # [k, q]: q >= k
    emask = np.where(c < r, 1.0, 0.0)                      # [k, q]: q < k
    msk = np.stack([dmask, dmask, emask, emask],
                   axis=1).reshape(P, 2, 2, P).astype(bf)  # [k, dm/em, hdup, q]

    f8 = ml_dtypes.float8_e4m3
    WS = 64.0  # weight scale: fp8 sweet spot; absorbed by RMSNorm (q/k) and
               # by the 64-valued ones-column of v_ext (v)

    def hi_lo(x):
        hi = x.astype(f8)
        lo = (x - hi.astype(np.float64)).astype(f8)
        return np.stack([hi, lo], axis=1)

    def pack_w(w_slice):
        # [O, H] -> lhsT [H, O] -> [128, 2(hl), 8(tp), 2(ti), O] fp8 x64
        wT = w_slice.T.astype(np.float64) * WS
        O = wT.shape[1]
        base = wT.reshape(NHT // 2, 2, P, O).transpose(2, 0, 1, 3)
        return np.ascontiguousarray(hi_lo(base))

    hs_packed = []
    for b in range(B):
        hsT = hidden_states[b].T.astype(np.float64)        # [H, S]
        # [p, sc, 2(hl), 8(tp), 2(ti), 4(q), s128] fp8 hi/lo
        hs6 = hsT.reshape(NHT // 2, 2, P, NCH, 4, P).transpose(2, 3, 0, 1, 4, 5)
        hi = hs6.astype(f8)
        lo = (hs6 - hi.astype(np.float64)).astype(f8)
        hs_packed.append(np.ascontiguousarray(np.stack([hi, lo], axis=2)))

    in_maps = []
    for core in range(8):
        b, g = divmod(core, 4)
        woT = wo[:, 512 * g:512 * (g + 1)].T.astype(np.float64)  # [512, H]
        wo_r = np.ascontiguousarray(
            woT.reshape(NQC, P, H).transpose(1, 0, 2)).astype(bf)
        in_maps.append({
            "hs": hs_packed[b],
            "wq": pack_w(wq[512 * g:512 * (g + 1), :]),
            "wk": pack_w(wk[256 * g:256 * (g + 1), :]),
            "wv": pack_w(wv[256 * g:256 * (g + 1), :]),
            "wo": wo_r,
            "tabs": tabs, "rots": rots, "msk": msk,
        })
    return in_maps


def _postprocess(results):
    out = np.empty((B, S, H), np.float32)
    for b in range(B):
        acc = np.zeros((H, S), np.float32)
        for g in range(4):
            y_r = results[4 * b + g]["y"].astype(np.float32)  # [128, 16, S]
            acc += y_r.transpose(1, 0, 2).reshape(H, S)
        out[b] = acc.T
    return out


def kernel(hidden_states, wq, wk, wv, wo, q_norm_weight, k_norm_weight):
    nc = _build_nc()
    in_maps = _host_inputs(hidden_states, wq, wk, wv, wo,
                           q_norm_weight, k_norm_weight)
    res = run_bass_kernel_spmd(nc, in_maps, list(range(8)))
    return _postprocess(res.results)


# revision 65
# speedup vs baseline: 1.0145x; 1.0145x over previous
"""Gemma3 sliding-window attention on 8 Trainium2 NeuronCores.

Sharding: core c handles batch b=c//4 and head-group g=c%4 (4 of 16 q heads,
2 of 8 kv heads). wq/wk/wv column-split, wo row-split; the 4 partial outputs
per batch are summed on host (no device collectives).

v2: all matmul operands in bf16 (fp32 PSUM accumulation), single-instruction
batched DMA loads from host-prepacked layouts, per-q-tile(128) attention with
the softmax denominator computed as a 129th V column in a [q,d]-oriented PV
matmul, XBAR DMA transposes to return attn to [d,q] for the output
projection, and software-pipelined instruction issue so the PE never waits
on the RMSNorm/RoPE vector chains.
"""

import math
import numpy as np
import ml_dtypes

import concourse.bacc as bacc
import concourse.mybir as mybir
import concourse.tile as tile
from concourse.bass_utils import run_bass_kernel_spmd

dt = mybir.dt
AFT = mybir.ActivationFunctionType
BF = dt.bfloat16
F32 = dt.float32

B, S, H = 2, 2048, 2048
NQC, NKVC, D = 4, 2, 128          # per-core heads
WIN = 1024
EPS = 1e-6
THETA = 10000.0
P = 128
SCP = 512                          # phase-1 seq chunk
NCH = S // SCP                     # 4
NHT = H // P                       # 16
NST = S // P                       # 16
WT = WIN // P                      # 8 (window in tiles)
LAG = 2                            # attention PV pipeline depth (pair units)

_CACHE = {}


def _groups_for(t0, u0):
    """k-tile groups for one q tile: runs of <=4 tiles, diagonal tile alone
    last (so its [128,128] exp/mask stays separate)."""
    ts = list(range(t0, u0 + 1))
    if len(ts) == 1:
        return [ts]
    body, diag = ts[:-1], ts[-1:]
    gs = [body[i:i + 4] for i in range(0, len(body), 4)]
    gs.append(diag)
    return gs


def _build_nc():
    if "nc" in _CACHE:
        return _CACHE["nc"]
    nc = bacc.Bacc("TRN2", target_bir_lowering=False, debug=False, num_devices=8)

    F8 = dt.float8e4
    DR = mybir.MatmulPerfMode.DoubleRow
    # hi/lo fp8 pairs: x ~= hi + lo to ~0.1% rms; DoubleRow matmuls run the
    # (hi,hi), (hi,lo), (lo,hi) cross terms at 0.5 cyc/row over ht-pairs.
    hs_d = nc.dram_tensor("hs", [P, NCH, 2, NHT // 2, 2, 4, P], F8,
                          kind="ExternalInput").ap()
    wq_d = nc.dram_tensor("wq", [P, 2, NHT // 2, 2, NQC * D], F8,
                          kind="ExternalInput").ap()
    wk_d = nc.dram_tensor("wk", [P, 2, NHT // 2, 2, NKVC * D], F8,
                          kind="ExternalInput").ap()
    wv_d = nc.dram_tensor("wv", [P, 2, NHT // 2, 2, NKVC * D], F8,
                          kind="ExternalInput").ap()
    wo_d = nc.dram_tensor("wo", [P, NQC, H], BF, kind="ExternalInput").ap()
    tabs_d = nc.dram_tensor("tabs", [P, 4, S], BF, kind="ExternalInput").ap()
    rots_d = nc.dram_tensor("rots", [P, 2, P], BF, kind="ExternalInput").ap()
    msk_d = nc.dram_tensor("msk", [P, 2, 2, P], BF, kind="ExternalInput").ap()
    y_d = nc.dram_tensor("y", [P, NHT, S], BF, kind="ExternalOutput").ap()

    with nc.allow_low_precision(reason="bf16 kernel; rel-err budget 2e-2"), \
         tile.TileContext(nc) as tc:
        with (
            tc.tile_pool(name="const", bufs=1) as cpool,
            tc.tile_pool(name="qkv", bufs=1) as qkv,
            tc.tile_pool(name="wts", bufs=1) as wts,
        ):
            msk_sb = cpool.tile([P, 2, 2, P], BF, tag="msk")
            rots_sb = cpool.tile([P, 2, P], BF, tag="rots")
            ones_sb = cpool.tile([P, P], BF, tag="ones")
            eps_sb = cpool.tile([P, 1], F32, tag="eps")
            nc.vector.memset(ones_sb[:], 1.0)
            nc.vector.memset(eps_sb[:], EPS)
            dm_sb = msk_sb[:, 0, :, :]
            em_sb = msk_sb[:, 1, :, :]

            # weight loads: wv first (v-projection is the startup filler work),
            # then wk (k heads run before q heads), wq, wo last-needed.
            wv_sb = wts.tile([P, 2, NHT // 2, 2, NKVC * D], F8, tag="wv")
            wk_sb = wts.tile([P, 2, NHT // 2, 2, NKVC * D], F8, tag="wk")
            wq_sb = wts.tile([P, 2, NHT // 2, 2, NQC * D], F8, tag="wq")
            wo_sb = wts.tile([P, NQC, H], BF, tag="wo")

            qn_sb = qkv.tile([P, NQC, S], BF, tag="qn")
            kn_sb = qkv.tile([P, NKVC, S], BF, tag="kn")
            v_sb = qkv.tile([P, NST, NKVC, D + 1], BF, tag="v")
            nc.vector.memset(v_sb[:, :, :, D:D + 1], 64.0)

            # ---------------- phase 1: QKV projections + RMSNorm + RoPE ----
            # per (chunk, head): PE proj chain -> Act copy -> DVE square /
            # rope muls; the sum-of-squares and rotation matmuls for head m
            # are issued after head m+1's projection chain so PE never waits.
            with (
                tc.tile_pool(name="hsp", bufs=2) as hsp,
                tc.tile_pool(name="tabp", bufs=2) as tabp,
                tc.tile_pool(name="cpp", bufs=4) as cpp,
                tc.tile_pool(name="t1", bufs=3) as t1p,
                tc.tile_pool(name="t2", bufs=3) as t2p,
                tc.tile_pool(name="t3", bufs=3) as t3p,
                tc.tile_pool(name="t4", bufs=3) as t4p,
                tc.tile_pool(name="t5", bufs=4) as t5p,
                tc.tile_pool(name="t6", bufs=2) as t6p,
                tc.tile_pool(name="pp", bufs=2, space="PSUM") as ppp,
                tc.tile_pool(name="prb", bufs=2, space="PSUM") as prbp,
                tc.tile_pool(name="pvb", bufs=2, space="PSUM") as pvbp,
                tc.tile_pool(name="psv", bufs=2, space="PSUM") as psvp,
            ):
                # heads order: k0, k1, q0..q3 (wk arrives before wq)
                HEADS = [("k", 0), ("k", 1), ("q", 0), ("q", 1), ("q", 2), ("q", 3)]
                pend = []  # deferred norm/rope finishes (2-deep pipeline)

                def proj_chain(out_ps, w_sb8, mcols, hs_t):
                    first = True
                    for wi, xi in ((0, 0), (0, 1), (1, 0)):
                        for tp in range(NHT // 2):
                            nc.tensor.matmul(
                                out_ps[:], w_sb8[:, wi, tp, :, mcols],
                                hs_t[:, xi, tp, :, :, :],
                                perf_mode=DR, start=first,
                                stop=(wi == 1 and tp == NHT // 2 - 1))
                            first = False

                def v_chain(out_ps, hs_t, ss):
                    first = True
                    for wi, xi in ((0, 0), (0, 1), (1, 0)):
                        for tp in range(NHT // 2):
                            nc.tensor.matmul(
                                out_ps[:], hs_t[:, xi, tp, :, ss, :],
                                wv_sb[:, wi, tp, :, :],
                                perf_mode=DR, start=first,
                                stop=(wi == 1 and tp == NHT // 2 - 1))
                            first = False

                def finish(st):
                    kind, m, pp, cp, u_t, s0, tab_t = st
                    sq = t1p.tile([P, SCP], BF, tag="sq")
                    nc.scalar.square(sq[:], cp[:])
                    rb = prbp.tile([P, SCP], F32, tag="rb")
                    rot = rots_sb[:, 0, :] if kind == "q" else rots_sb[:, 1, :]
                    nc.tensor.matmul(rb[:], rot, cp[:], start=True, stop=True)
                    vb = pvbp.tile([P, SCP], F32, tag="vb")
                    nc.tensor.matmul(vb[:], ones_sb[:], sq[:], start=True, stop=True)
                    sd = t2p.tile([P, SCP], F32, tag="sd")
                    nc.scalar.activation(sd[:], vb[:], AFT.Sqrt, bias=eps_sb[:],
                                         scale=1.0 / D)
                    inv = t3p.tile([P, SCP], BF, tag="inv")
                    nc.vector.reciprocal(inv[:], sd[:])
                    # tsin: rb (PSUM) is ready late
                    tsin = t4p.tile([P, SCP], BF, tag="tsin")
                    sin_t = tab_t[:, 1 if kind == "q" else 3, :]
                    nc.vector.tensor_mul(tsin[:], rb[:], sin_t)
                    nc.vector.tensor_add(u_t[:], u_t[:], tsin[:])
                    dst = qn_sb if kind == "q" else kn_sb
                    nc.vector.tensor_mul(dst[:, m, s0:s0 + SCP], u_t[:], inv[:])

                for sc in range(NCH):
                    s0 = sc * SCP
                    hs_sb = hsp.tile([P, 2, NHT // 2, 2, 4, P], F8, tag="hs")
                    if sc == 0:
                        # startup-critical order: hi parts first (the hi-hi
                        # chain leads each accumulation), v before k/q.
                        nc.sync.dma_start(out=wv_sb[:, 0], in_=wv_d[:, 0])
                        nc.sync.dma_start(out=hs_sb[:, 0, 0:4], in_=hs_d[:, 0, 0, 0:4])
                        nc.sync.dma_start(out=hs_sb[:, 0, 4:8], in_=hs_d[:, 0, 0, 4:8])
                        nc.sync.dma_start(out=wv_sb[:, 1], in_=wv_d[:, 1])
                        nc.sync.dma_start(out=hs_sb[:, 1], in_=hs_d[:, 0, 1])
                        nc.sync.dma_start(out=wk_sb[:], in_=wk_d[:])
                        nc.sync.dma_start(out=msk_sb[:], in_=msk_d[:])
                        nc.sync.dma_start(out=rots_sb[:], in_=rots_d[:])
                        nc.sync.dma_start(out=wq_sb[:], in_=wq_d[:])
                    else:
                        nc.sync.dma_start(out=hs_sb[:], in_=hs_d[:, sc])
                    tab_sb = tabp.tile([P, 4, SCP], BF, tag="tab")
                    nc.sync.dma_start(out=tab_sb[:], in_=tabs_d[:, :, s0:s0 + SCP])
                    if sc == 0:
                        nc.sync.dma_start(out=wo_sb[:], in_=wo_d[:])

                    # v projection: natural [seq, d] layout + filler work
                    for ss in range(SCP // P):
                        st_g = sc * (SCP // P) + ss
                        pv = psvp.tile([P, NKVC * D], F32, tag="pv")
                        v_chain(pv, hs_sb, ss)
                        nc.vector.tensor_copy(v_sb[:, st_g, :, 0:D], pv[:])
                        if ss == 0 and pend:
                            # finish the previous chunk's last head here: its
                            # DVE ops land ahead of the remaining v copies
                            finish(pend.pop(0))

                    for kind, m in HEADS:
                        w_sb = wq_sb if kind == "q" else wk_sb
                        pp = ppp.tile([P, SCP], F32, tag="pp")
                        proj_chain(pp, w_sb, slice(m * D, (m + 1) * D), hs_sb)
                        cp = cpp.tile([P, SCP], BF, tag="cp")
                        nc.scalar.copy(cp[:], pp[:])
                        u_t = t5p.tile([P, SCP], BF, tag="u")
                        cos_t = tab_sb[:, 0 if kind == "q" else 2, :]
                        nc.vector.tensor_mul(u_t[:], cp[:], cos_t)
                        pend.append((kind, m, pp, cp, u_t, s0, tab_sb))
                        if len(pend) > 1:
                            finish(pend.pop(0))
                for st in pend:
                    finish(st)

            # ---------------- phase 2+3: attention + output projection -----
            with (
                tc.tile_pool(name="pb", bufs=LAG + 2) as pbp,
                tc.tile_pool(name="invp", bufs=4) as invp,
                tc.tile_pool(name="aq", bufs=2) as aqp,
                tc.tile_pool(name="aT", bufs=2) as aTp,
                tc.tile_pool(name="ysb", bufs=2) as ysp,
                tc.tile_pool(name="psc", bufs=4, space="PSUM") as pscp,
                tc.tile_pool(name="pa", bufs=2, space="PSUM") as pap,
                tc.tile_pool(name="psy", bufs=2, space="PSUM") as psyp,
            ):
                queue = []
                slab_tiles = {}

                def emit_scores(kvh, u0):
                    # paired unit: both q heads of this kv head at once
                    h0 = 2 * kvh
                    t0 = max(0, u0 - WT)
                    n = u0 - t0 + 1
                    p_t = pbp.tile([P, WT + 1, 2, P], BF, tag="p", name="p_t")
                    qn_sl = qn_sb[:, h0:h0 + 2, u0 * P:(u0 + 1) * P]
                    for g0 in range(0, n, 2):
                        gn = min(2, n - g0)
                        sc_t = pscp.tile([P, 2, 2, P], F32, tag="sc",
                                         name="sc_t")
                        for i in range(gn):
                            t = t0 + g0 + i
                            nc.tensor.matmul(
                                sc_t[:, i, :, :],
                                kn_sb[:, kvh, t * P:(t + 1) * P],
                                qn_sl, start=True, stop=True)
                        nc.scalar.activation(p_t[:, g0:g0 + gn, :, :],
                                             sc_t[:, 0:gn, :, :], AFT.Exp)
                        for i in range(gn):
                            t = t0 + g0 + i
                            if t == u0:
                                blk = p_t[:, g0 + i, :, :]
                                nc.vector.tensor_mul(blk, blk, dm_sb)
                            elif u0 >= WT and t == u0 - WT:
                                blk = p_t[:, g0 + i, :, :]
                                nc.vector.tensor_mul(blk, blk, em_sb)
                    return (kvh, u0, t0, p_t)

                def emit_pv(st):
                    kvh, u0, t0, p_t = st
                    n = u0 - t0 + 1
                    slab = slab_tiles[u0 // 4]
                    for hh in range(2):
                        h = 2 * kvh + hh
                        a_t = pap.tile([P, D + 1], F32, tag="a")
                        for i in range(n):
                            nc.tensor.matmul(
                                a_t[:], p_t[:, i, hh, :],
                                v_sb[:, t0 + i, kvh, :],
                                start=(i == 0), stop=(i == n - 1))
                        inv = invp.tile([P, 1], F32, tag="inv")
                        nc.vector.reciprocal(inv[:], a_t[:, D:D + 1])
                        nc.vector.tensor_scalar_mul(slab[:, h, u0 % 4, :],
                                                    a_t[:, 0:D], inv[:])

                def emit_transpose(s4, h):
                    # Act HWDGE queue: avoids head-of-line blocking behind the
                    # SP queue's y-out DMAs (which wait on DVE copies). Issued
                    # one per unit so the Act SEQ time (~0.7us per DMA) does
                    # not delay exp dispatch in a lump.
                    if h == 0:
                        aT = aTp.tile([P, NQC, 4, P], BF, tag="aT", name="aT")
                        slab_tiles[("T", s4)] = aT
                    aT = slab_tiles[("T", s4)]
                    slab = slab_tiles[s4]
                    nc.sync.dma_start_transpose(out=aT[:, h, :, :],
                                                in_=slab[:, h, :, :])

                op_queue = []  # (s4, mo) outproj chains, spread across units
                op_state = {}

                def emit_outproj_chain():
                    if not op_queue:
                        return
                    s4, mo = op_queue.pop(0)
                    aT = slab_tiles[("T", s4)]
                    mog, mo4 = divmod(mo, 4)
                    if mo4 == 0:
                        op_state["y"] = ysp.tile([P, 4, SCP], BF, tag="y",
                                                 name="y_t")
                    y_t = op_state["y"]
                    yp = psyp.tile([P, SCP], F32, tag="yp")
                    for h in range(NQC):
                        nc.tensor.matmul(
                            yp[:], wo_sb[:, h, mo * P:(mo + 1) * P],
                            aT[:, h, :, :],
                            start=(h == 0), stop=(h == NQC - 1))
                    nc.vector.tensor_copy(y_t[:, mo4, :], yp[:])
                    if mo4 == 3:
                        nc.sync.dma_start(
                            out=y_d[:, mog * 4:(mog + 1) * 4,
                                    s4 * SCP:(s4 + 1) * SCP],
                            in_=y_t[:])

                def emit_outproj(s4):
                    op_queue.extend((s4, mo) for mo in range(NHT))

                # descending u0: the big steady-state units come first and
                # prime the PV pipeline; the small ramp units land at the end
                # where the outproj slabs provide PE filler work.
                for u0 in range(NST - 1, -1, -1):
                    if u0 % 4 == 3:
                        slab_tiles[u0 // 4] = aqp.tile([P, NQC, 4, P], BF,
                                                       tag="aq", name="aq")
                    for kvh in range(NKVC):
                        queue.append(emit_scores(kvh, u0))
                        if len(queue) > LAG:
                            emit_pv(queue.pop(0))
                        if u0 == 0 and queue:
                            emit_pv(queue.pop(0))  # drain early for the tail
                        if u0 % 4 == 2 and u0 <= NST - 6:
                            emit_transpose(u0 // 4 + 1, 2 * kvh)
                            emit_transpose(u0 // 4 + 1, 2 * kvh + 1)
                        emit_outproj_chain()
                        if u0 > 1:
                            emit_outproj_chain()  # hold chains back for the
                            # tail drain (fills the final transpose latency)
                    if u0 % 4 == 2 and u0 <= NST - 6:
                        emit_outproj(u0 // 4 + 1)
                while queue:
                    emit_pv(queue.pop(0))
                    emit_outproj_chain()
                    emit_outproj_chain()
                # first slab (last processed): split transposes across both
                # HWDGE queues to halve the serial latency in the tail
                aT = aTp.tile([P, NQC, 4, P], BF, tag="aT", name="aT")
                slab_tiles[("T", 0)] = aT
                slab = slab_tiles[0]
                for h in range(NQC):
                    eng = nc.scalar if h % 2 == 0 else nc.sync
                    eng.dma_start_transpose(out=aT[:, h, :, :],
                                            in_=slab[:, h, :, :])
                emit_outproj(0)
                while op_queue:
                    emit_outproj_chain()

    nc.compile()
    _CACHE["nc"] = nc
    return nc


def _host_inputs(hidden_states, wq, wk, wv, wo, q_norm_weight, k_norm_weight):
    """Per-core input dicts (8 cores: c = 4*b + g)."""
    bf = ml_dtypes.bfloat16
    f = np.float32
    scale = 1.0 / math.sqrt(D)
    inv_freq = 1.0 / (THETA ** (np.arange(0, D, 2, dtype=np.float64) / D))
    t = np.arange(S, dtype=np.float64)
    freqs = np.outer(t, inv_freq)
    emb = np.concatenate([freqs, freqs], axis=-1)          # [S, D]
    cosT = np.cos(emb).T.astype(np.float64)                # [D, S]
    sinT = np.sin(emb).T.astype(np.float64)
    qw = (1.0 + q_norm_weight).astype(np.float64)
    kw = (1.0 + k_norm_weight).astype(np.float64)

    # rotate-half matrices with norm weights folded (lhsT layout, like
    # baseline): rb = rots.T @ x = (R * w) @ x
    R = np.zeros((D, D), np.float64)
    hh = D // 2
    for i in range(hh):
        R[i, i + hh] = -1.0
        R[i + hh, i] = 1.0
    rqT = np.ascontiguousarray((R * qw[None, :]).T)
    rkT = np.ascontiguousarray((R * kw[None, :]).T)
    rots = np.stack([rqT, rkT], axis=1).astype(bf)         # [D, 2, D]

    tabs = np.stack([
        cosT * qw[:, None] * scale,
        sinT * scale,
        cosT * kw[:, None],
        sinT,
    ], axis=1).astype(bf)                                  # [D, 4, S]

    r = np.arange(P)[:, None]
    c = np.arange(P)[None, :]
    dmask = np.where(c >= r, 1.0, 0.0)                     # [k, q]: q >= k
    emask = np.where(c < r, 1.0, 0.0)                      # [k, q]: q < k
    msk = np.stack([dmask, dmask, emask, emask],
                   axis=1).reshape(P, 2, 2, P).astype(bf)  # [k, dm/em, hdup, q]

    f8 = ml_dtypes.float8_e4m3
    WS = 64.0  # weight scale: fp8 sweet spot; absorbed by RMSNorm (q/k) and
               # by the 64-valued ones-column of v_ext (v)

    def hi_lo(x):
        hi = x.astype(f8)
        lo = (x - hi.astype(np.float64)).astype(f8)
        return np.stack([hi, lo], axis=1)

    def pack_w(w_slice):
        # [O, H] -> lhsT [H, O] -> [128, 2(hl), 8(tp), 2(ti), O] fp8 x64
        wT = w_slice.T.astype(np.float64) * WS
        O = wT.shape[1]
        base = wT.reshape(NHT // 2, 2, P, O).transpose(2, 0, 1, 3)
        return np.ascontiguousarray(hi_lo(base))

    hs_packed = []
    for b in range(B):
        hsT = hidden_states[b].T.astype(np.float64)        # [H, S]
        # [p, sc, 2(hl), 8(tp), 2(ti), 4(q), s128] fp8 hi/lo
        hs6 = hsT.reshape(NHT // 2, 2, P, NCH, 4, P).transpose(2, 3, 0, 1, 4, 5)
        hi = hs6.astype(f8)
        lo = (hs6 - hi.astype(np.float64)).astype(f8)
        hs_packed.append(np.ascontiguousarray(np.stack([hi, lo], axis=2)))

    in_maps = []
    for core in range(8):
        b, g = divmod(core, 4)
        woT = wo[:, 512 * g:512 * (g + 1)].T.astype(np.float64)  # [512, H]
        wo_r = np.ascontiguousarray(
            woT.reshape(NQC, P, H).transpose(1, 0, 2)).astype(bf)
        in_maps.append({
            "hs": hs_packed[b],
            "wq": pack_w(wq[512 * g:512 * (g + 1), :]),
            "wk": pack_w(wk[256 * g:256 * (g + 1), :]),
            "wv": pack_w(wv[256 * g:256 * (g + 1), :]),
            "wo": wo_r,
            "tabs": tabs, "rots": rots, "msk": msk,
        })
    return in_maps


def _postprocess(results):
    out = np.empty((B, S, H), np.float32)
    for b in range(B):
        acc = np.zeros((H, S), np.float32)
        for g in range(4):
            y_r = results[4 * b + g]["y"].astype(np.float32)  # [128, 16, S]
            acc += y_r.transpose(1, 0, 2).reshape(H, S)
        out[b] = acc.T
    return out


def kernel(hidden_states, wq, wk, wv, wo, q_norm_weight, k_norm_weight):
    nc = _build_nc()
    in_maps = _host_inputs(hidden_states, wq, wk, wv, wo,
                           q_norm_weight, k_norm_weight)
    res = run_bass_kernel_spmd(nc, in_maps, list(range(8)))
    return _postprocess(res.results)


# revision 80
# speedup vs baseline: 1.0296x; 1.0149x over previous
"""Gemma3 sliding-window attention on 8 Trainium2 NeuronCores.

Sharding: core c handles batch b=c//4 and head-group g=c%4 (4 of 16 q heads,
2 of 8 kv heads). wq/wk/wv column-split, wo row-split; the 4 partial outputs
per batch are summed on host (no device collectives).

v2: all matmul operands in bf16 (fp32 PSUM accumulation), single-instruction
batched DMA loads from host-prepacked layouts, per-q-tile(128) attention with
the softmax denominator computed as a 129th V column in a [q,d]-oriented PV
matmul, XBAR DMA transposes to return attn to [d,q] for the output
projection, and software-pipelined instruction issue so the PE never waits
on the RMSNorm/RoPE vector chains.
"""

import math
import numpy as np
import ml_dtypes

import concourse.bacc as bacc
import concourse.mybir as mybir
import concourse.tile as tile
from concourse.bass_utils import run_bass_kernel_spmd

dt = mybir.dt
AFT = mybir.ActivationFunctionType
BF = dt.bfloat16
F32 = dt.float32

B, S, H = 2, 2048, 2048
NQC, NKVC, D = 4, 2, 128          # per-core heads
WIN = 1024
EPS = 1e-6
THETA = 10000.0
P = 128
SCP = 512                          # phase-1 seq chunk
NCH = S // SCP                     # 4
NHT = H // P                       # 16
NST = S // P                       # 16
WT = WIN // P                      # 8 (window in tiles)
LAG = 2                            # attention PV pipeline depth (pair units)

_CACHE = {}


def _groups_for(t0, u0):
    """k-tile groups for one q tile: runs of <=4 tiles, diagonal tile alone
    last (so its [128,128] exp/mask stays separate)."""
    ts = list(range(t0, u0 + 1))
    if len(ts) == 1:
        return [ts]
    body, diag = ts[:-1], ts[-1:]
    gs = [body[i:i + 4] for i in range(0, len(body), 4)]
    gs.append(diag)
    return gs


def _build_nc():
    if "nc" in _CACHE:
        return _CACHE["nc"]
    nc = bacc.Bacc("TRN2", target_bir_lowering=False, debug=False, num_devices=8)

    F8 = dt.float8e4
    DR = mybir.MatmulPerfMode.DoubleRow
    # hi/lo fp8 pairs: x ~= hi + lo to ~0.1% rms; DoubleRow matmuls run the
    # (hi,hi), (hi,lo), (lo,hi) cross terms at 0.5 cyc/row over ht-pairs.
    hs_d = nc.dram_tensor("hs", [P, NCH, 2, NHT // 2, 2, 4, P], F8,
                          kind="ExternalInput").ap()
    wq_d = nc.dram_tensor("wq", [P, 2, NHT // 2, 2, NQC * D], F8,
                          kind="ExternalInput").ap()
    wk_d = nc.dram_tensor("wk", [P, 2, NHT // 2, 2, NKVC * D], F8,
                          kind="ExternalInput").ap()
    wv_d = nc.dram_tensor("wv", [P, 2, NHT // 2, 2, NKVC * D], F8,
                          kind="ExternalInput").ap()
    wo_d = nc.dram_tensor("wo", [P, NQC, H], BF, kind="ExternalInput").ap()
    tabs_d = nc.dram_tensor("tabs", [P, 4, S], BF, kind="ExternalInput").ap()
    msk_d = nc.dram_tensor("msk", [P, 2, 2, P], BF, kind="ExternalInput").ap()
    y_d = nc.dram_tensor("y", [P, NHT, S], BF, kind="ExternalOutput").ap()

    with nc.allow_low_precision(reason="bf16 kernel; rel-err budget 2e-2"), \
         tile.TileContext(nc) as tc:
        with (
            tc.tile_pool(name="const", bufs=1) as cpool,
            tc.tile_pool(name="qkv", bufs=1) as qkv,
            tc.tile_pool(name="wts", bufs=1) as wts,
        ):
            msk_sb = cpool.tile([P, 2, 2, P], BF, tag="msk")
            ones_sb = cpool.tile([P, P], BF, tag="ones")
            eps_sb = cpool.tile([P, 1], F32, tag="eps")
            nc.vector.memset(ones_sb[:], 1.0)
            nc.vector.memset(eps_sb[:], EPS)
            dm_sb = msk_sb[:, 0, :, :]
            em_sb = msk_sb[:, 1, :, :]

            # weight loads: wv first (v-projection is the startup filler work),
            # then wk (k heads run before q heads), wq, wo last-needed.
            wv_sb = wts.tile([P, 2, NHT // 2, 2, NKVC * D], F8, tag="wv")
            wk_sb = wts.tile([P, 2, NHT // 2, 2, NKVC * D], F8, tag="wk")
            wq_sb = wts.tile([P, 2, NHT // 2, 2, NQC * D], F8, tag="wq")
            wo_sb = wts.tile([P, NQC, H], BF, tag="wo")

            qn_sb = qkv.tile([P, NQC, S], BF, tag="qn")
            kn_sb = qkv.tile([P, NKVC, S], BF, tag="kn")
            v_sb = qkv.tile([P, NST, NKVC, D + 1], BF, tag="v")
            nc.vector.memset(v_sb[:, :, :, D:D + 1], 64.0)

            # ---------------- phase 1: QKV projections + RMSNorm + RoPE ----
            # per (chunk, head): PE proj chain -> Act copy -> DVE square /
            # rope muls; the sum-of-squares and rotation matmuls for head m
            # are issued after head m+1's projection chain so PE never waits.
            with (
                tc.tile_pool(name="hsp", bufs=2) as hsp,
                tc.tile_pool(name="tabp", bufs=2) as tabp,
                tc.tile_pool(name="cpp", bufs=4) as cpp,
                tc.tile_pool(name="t1", bufs=3) as t1p,
                tc.tile_pool(name="t2", bufs=3) as t2p,
                tc.tile_pool(name="t3", bufs=3) as t3p,
                tc.tile_pool(name="t4", bufs=3) as t4p,
                tc.tile_pool(name="t5", bufs=4) as t5p,
                tc.tile_pool(name="t6", bufs=3) as t6p,
                tc.tile_pool(name="pp", bufs=3, space="PSUM") as ppp,
                tc.tile_pool(name="pvb", bufs=2, space="PSUM") as pvbp,
                tc.tile_pool(name="psv", bufs=3, space="PSUM") as psvp,
            ):
                # heads order: k0, k1, q0..q3 (wk arrives before wq)
                HEADS = [("k", 0), ("k", 1), ("q", 0), ("q", 1), ("q", 2), ("q", 3)]
                pend = []  # deferred norm/rope finishes (2-deep pipeline)

                def proj_chain(out_ps, w_sb8, mcols, hs_t):
                    first = True
                    for wi, xi in ((0, 0), (0, 1), (1, 0)):
                        for tp in range(NHT // 2):
                            nc.tensor.matmul(
                                out_ps[:], w_sb8[:, wi, tp, :, mcols],
                                hs_t[:, xi, tp, :, :, :],
                                perf_mode=DR, start=first,
                                stop=(wi == 1 and tp == NHT // 2 - 1))
                            first = False

                def v_chain(out_ps, hs_t, ss):
                    first = True
                    for wi, xi in ((0, 0), (0, 1), (1, 0)):
                        for tp in range(NHT // 2):
                            nc.tensor.matmul(
                                out_ps[:], hs_t[:, xi, tp, :, ss, :],
                                wv_sb[:, wi, tp, :, :],
                                perf_mode=DR, start=first,
                                stop=(wi == 1 and tp == NHT // 2 - 1))
                            first = False

                def finish(st):
                    kind, m, pp, cp, u_t, s0, tab_t, rot = st
                    sq = t1p.tile([P, SCP], BF, tag="sq")
                    nc.scalar.square(sq[:], cp[:])
                    vb = pvbp.tile([P, SCP], F32, tag="vb")
                    nc.tensor.matmul(vb[:], ones_sb[:], sq[:], start=True, stop=True)
                    sd = t2p.tile([P, SCP], F32, tag="sd")
                    nc.scalar.activation(sd[:], vb[:], AFT.Sqrt, bias=eps_sb[:],
                                         scale=1.0 / D)
                    inv = t3p.tile([P, SCP], BF, tag="inv")
                    nc.vector.reciprocal(inv[:], sd[:])
                    tsin = t4p.tile([P, SCP], BF, tag="tsin")
                    sin_t = tab_t[:, 1 if kind == "q" else 3, :]
                    nc.vector.tensor_mul(tsin[:], rot[:], sin_t)
                    nc.vector.tensor_add(u_t[:], u_t[:], tsin[:])
                    dst = qn_sb if kind == "q" else kn_sb
                    nc.vector.tensor_mul(dst[:, m, s0:s0 + SCP], u_t[:], inv[:])

                chunk_tiles = {}

                def load_chunk(sc):
                    s0 = sc * SCP
                    hs_sb = hsp.tile([P, 2, NHT // 2, 2, 4, P], F8, tag="hs",
                                     name="hs_sb")
                    nc.sync.dma_start(out=hs_sb[:], in_=hs_d[:, sc])
                    tab_sb = tabp.tile([P, 4, SCP], BF, tag="tab",
                                       name="tab_sb")
                    nc.sync.dma_start(out=tab_sb[:],
                                      in_=tabs_d[:, :, s0:s0 + SCP])
                    chunk_tiles[sc] = (hs_sb, tab_sb)

                for sc in range(NCH):
                    s0 = sc * SCP
                    if sc > 0:
                        hs_sb, tab_sb = chunk_tiles[sc]
                    else:
                        hs_sb = hsp.tile([P, 2, NHT // 2, 2, 4, P], F8,
                                         tag="hs", name="hs_sb")
                    if sc == 0:
                        # startup-critical order: hi parts first (the hi-hi
                        # chain leads each accumulation), v before k/q.
                        nc.sync.dma_start(out=wv_sb[:, 0], in_=wv_d[:, 0])
                        nc.sync.dma_start(out=hs_sb[:, 0, 0:4], in_=hs_d[:, 0, 0, 0:4])
                        nc.sync.dma_start(out=hs_sb[:, 0, 4:8], in_=hs_d[:, 0, 0, 4:8])
                        nc.sync.dma_start(out=wv_sb[:, 1], in_=wv_d[:, 1])
                        nc.sync.dma_start(out=hs_sb[:, 1], in_=hs_d[:, 0, 1])
                        nc.sync.dma_start(out=wk_sb[:], in_=wk_d[:])
                        nc.sync.dma_start(out=msk_sb[:], in_=msk_d[:])
                        nc.sync.dma_start(out=wq_sb[:], in_=wq_d[:])
                    if sc == 0:
                        tab_sb = tabp.tile([P, 4, SCP], BF, tag="tab",
                                           name="tab_sb")
                        nc.sync.dma_start(out=tab_sb[:],
                                          in_=tabs_d[:, :, s0:s0 + SCP])
                        nc.sync.dma_start(out=wo_sb[:], in_=wo_d[:])

                    # v projection: natural [seq, d] layout + filler work
                    for ss in range(SCP // P):
                        st_g = sc * (SCP // P) + ss
                        pv = psvp.tile([P, NKVC * D], F32, tag="pv")
                        v_chain(pv, hs_sb, ss)
                        nc.vector.tensor_copy(v_sb[:, st_g, :, 0:D], pv[:])
                        if ss == 0 and pend:
                            # finish the previous chunk's last head here: its
                            # DVE ops land ahead of the remaining v copies
                            finish(pend.pop(0))
                        if ss == 0 and sc + 1 < NCH:
                            load_chunk(sc + 1)  # prefetch ahead of rot DMAs

                    for kind, m in HEADS:
                        w_sb = wq_sb if kind == "q" else wk_sb
                        pp = ppp.tile([P, SCP], F32, tag="pp")
                        proj_chain(pp, w_sb, slice(m * D, (m + 1) * D), hs_sb)
                        cp = cpp.tile([P, SCP], BF, tag="cp")
                        nc.scalar.copy(cp[:], pp[:])
                        # rotate_half as a partition-shift DMA (sign and norm
                        # weight are folded into the sin tables on the host)
                        rot = t6p.tile([P, SCP], BF, tag="rot")
                        nc.sync.dma_start(out=rot[0:64, :], in_=cp[64:128, :])
                        nc.sync.dma_start(out=rot[64:128, :], in_=cp[0:64, :])
                        u_t = t5p.tile([P, SCP], BF, tag="u")
                        cos_t = tab_sb[:, 0 if kind == "q" else 2, :]
                        nc.vector.tensor_mul(u_t[:], cp[:], cos_t)
                        pend.append((kind, m, pp, cp, u_t, s0, tab_sb, rot))
                        if len(pend) > 1:
                            finish(pend.pop(0))
                for st in pend:
                    finish(st)

            # ---------------- phase 2+3: attention + output projection -----
            with (
                tc.tile_pool(name="pb", bufs=LAG + 2) as pbp,
                tc.tile_pool(name="invp", bufs=4) as invp,
                tc.tile_pool(name="aq", bufs=2) as aqp,
                tc.tile_pool(name="aT", bufs=2) as aTp,
                tc.tile_pool(name="ysb", bufs=2) as ysp,
                tc.tile_pool(name="psc", bufs=4, space="PSUM") as pscp,
                tc.tile_pool(name="pa", bufs=2, space="PSUM") as pap,
                tc.tile_pool(name="psy", bufs=2, space="PSUM") as psyp,
            ):
                queue = []
                slab_tiles = {}

                def emit_scores(kvh, u0):
                    # paired unit: both q heads of this kv head at once
                    h0 = 2 * kvh
                    t0 = max(0, u0 - WT)
                    n = u0 - t0 + 1
                    p_t = pbp.tile([P, WT + 1, 2, P], BF, tag="p", name="p_t")
                    qn_sl = qn_sb[:, h0:h0 + 2, u0 * P:(u0 + 1) * P]
                    for g0 in range(0, n, 2):
                        gn = min(2, n - g0)
                        sc_t = pscp.tile([P, 2, 2, P], F32, tag="sc",
                                         name="sc_t")
                        for i in range(gn):
                            t = t0 + g0 + i
                            nc.tensor.matmul(
                                sc_t[:, i, :, :],
                                kn_sb[:, kvh, t * P:(t + 1) * P],
                                qn_sl, start=True, stop=True)
                        nc.scalar.activation(p_t[:, g0:g0 + gn, :, :],
                                             sc_t[:, 0:gn, :, :], AFT.Exp)
                        for i in range(gn):
                            t = t0 + g0 + i
                            if t == u0:
                                blk = p_t[:, g0 + i, :, :]
                                nc.vector.tensor_mul(blk, blk, dm_sb)
                            elif u0 >= WT and t == u0 - WT:
                                blk = p_t[:, g0 + i, :, :]
                                nc.vector.tensor_mul(blk, blk, em_sb)
                    return (kvh, u0, t0, p_t)

                def emit_pv(st):
                    kvh, u0, t0, p_t = st
                    n = u0 - t0 + 1
                    slab = slab_tiles[u0 // 4]
                    for hh in range(2):
                        h = 2 * kvh + hh
                        a_t = pap.tile([P, D + 1], F32, tag="a")
                        for i in range(n):
                            nc.tensor.matmul(
                                a_t[:], p_t[:, i, hh, :],
                                v_sb[:, t0 + i, kvh, :],
                                start=(i == 0), stop=(i == n - 1))
                        inv = invp.tile([P, 1], F32, tag="inv")
                        nc.vector.reciprocal(inv[:], a_t[:, D:D + 1])
                        nc.vector.tensor_scalar_mul(slab[:, h, u0 % 4, :],
                                                    a_t[:, 0:D], inv[:])

                def emit_transpose(s4, h):
                    # Act HWDGE queue: avoids head-of-line blocking behind the
                    # SP queue's y-out DMAs (which wait on DVE copies). Issued
                    # one per unit so the Act SEQ time (~0.7us per DMA) does
                    # not delay exp dispatch in a lump.
                    if h == 0:
                        aT = aTp.tile([P, NQC, 4, P], BF, tag="aT", name="aT")
                        slab_tiles[("T", s4)] = aT
                    aT = slab_tiles[("T", s4)]
                    slab = slab_tiles[s4]
                    nc.sync.dma_start_transpose(out=aT[:, h, :, :],
                                                in_=slab[:, h, :, :])

                op_queue = []  # (s4, mo) outproj chains, spread across units
                op_state = {}

                def emit_outproj_chain():
                    if not op_queue:
                        return
                    s4, mo = op_queue.pop(0)
                    aT = slab_tiles[("T", s4)]
                    mog, mo4 = divmod(mo, 4)
                    if mo4 == 0:
                        op_state["y"] = ysp.tile([P, 4, SCP], BF, tag="y",
                                                 name="y_t")
                    y_t = op_state["y"]
                    yp = psyp.tile([P, SCP], F32, tag="yp")
                    for h in range(NQC):
                        nc.tensor.matmul(
                            yp[:], wo_sb[:, h, mo * P:(mo + 1) * P],
                            aT[:, h, :, :],
                            start=(h == 0), stop=(h == NQC - 1))
                    nc.vector.tensor_copy(y_t[:, mo4, :], yp[:])
                    if mo4 == 3:
                        nc.sync.dma_start(
                            out=y_d[:, mog * 4:(mog + 1) * 4,
                                    s4 * SCP:(s4 + 1) * SCP],
                            in_=y_t[:])

                def emit_outproj(s4):
                    op_queue.extend((s4, mo) for mo in range(NHT))

                # descending u0: the big steady-state units come first and
                # prime the PV pipeline; the small ramp units land at the end
                # where the outproj slabs provide PE filler work.
                for u0 in range(NST - 1, -1, -1):
                    if u0 % 4 == 3:
                        slab_tiles[u0 // 4] = aqp.tile([P, NQC, 4, P], BF,
                                                       tag="aq", name="aq")
                    for kvh in range(NKVC):
                        queue.append(emit_scores(kvh, u0))
                        if len(queue) > LAG:
                            emit_pv(queue.pop(0))
                        if u0 == 0 and queue:
                            emit_pv(queue.pop(0))  # drain early for the tail
                        if u0 % 4 == 2 and u0 <= NST - 6:
                            emit_transpose(u0 // 4 + 1, 2 * kvh)
                            emit_transpose(u0 // 4 + 1, 2 * kvh + 1)
                        emit_outproj_chain()
                        if u0 > 1:
                            emit_outproj_chain()  # hold chains back for the
                            # tail drain (fills the final transpose latency)
                    if u0 % 4 == 2 and u0 <= NST - 6:
                        emit_outproj(u0 // 4 + 1)
                while queue:
                    emit_pv(queue.pop(0))
                    emit_outproj_chain()
                    emit_outproj_chain()
                # first slab (last processed): split transposes across both
                # HWDGE queues to halve the serial latency in the tail
                aT = aTp.tile([P, NQC, 4, P], BF, tag="aT", name="aT")
                slab_tiles[("T", 0)] = aT
                slab = slab_tiles[0]
                for h in range(NQC):
                    eng = nc.scalar if h % 2 == 0 else nc.sync
                    eng.dma_start_transpose(out=aT[:, h, :, :],
                                            in_=slab[:, h, :, :])
                emit_outproj(0)
                while op_queue:
                    emit_outproj_chain()

    nc.compile()
    _CACHE["nc"] = nc
    return nc


def _host_inputs(hidden_states, wq, wk, wv, wo, q_norm_weight, k_norm_weight):
    """Per-core input dicts (8 cores: c = 4*b + g)."""
    bf = ml_dtypes.bfloat16
    f = np.float32
    scale = 1.0 / math.sqrt(D)
    inv_freq = 1.0 / (THETA ** (np.arange(0, D, 2, dtype=np.float64) / D))
    t = np.arange(S, dtype=np.float64)
    freqs = np.outer(t, inv_freq)
    emb = np.concatenate([freqs, freqs], axis=-1)          # [S, D]
    cosT = np.cos(emb).T.astype(np.float64)                # [D, S]
    sinT = np.sin(emb).T.astype(np.float64)
    qw = (1.0 + q_norm_weight).astype(np.float64)
    kw = (1.0 + k_norm_weight).astype(np.float64)

    # rotate_half is done on-device as a plain partition-shift DMA
    # (rot[d] = x[(d+64)%128]); the rotation sign and the SHIFTED norm
    # weight are folded into the sin tables here:
    #   sin_eff[d] = sgn(d) * sin[d] * w[(d+64)%128],  sgn = -1 for d<64
    hh = D // 2
    sgn = np.where(np.arange(D) < hh, -1.0, 1.0)
    qw_sh = np.roll(qw, -hh)   # w[(d+64)%128]
    kw_sh = np.roll(kw, -hh)
    tabs = np.stack([
        cosT * qw[:, None] * scale,
        sinT * (sgn * qw_sh)[:, None] * scale,
        cosT * kw[:, None],
        sinT * (sgn * kw_sh)[:, None],
    ], axis=1).astype(bf)                                  # [D, 4, S]

    r = np.arange(P)[:, None]
    c = np.arange(P)[None, :]
    dmask = np.where(c >= r, 1.0, 0.0)                     # [k, q]: q >= k
    emask = np.where(c < r, 1.0, 0.0)                      # [k, q]: q < k
    msk = np.stack([dmask, dmask, emask, emask],
                   axis=1).reshape(P, 2, 2, P).astype(bf)  # [k, dm/em, hdup, q]

    f8 = ml_dtypes.float8_e4m3
    WS = 64.0  # weight scale: fp8 sweet spot; absorbed by RMSNorm (q/k) and
               # by the 64-valued ones-column of v_ext (v)

    def hi_lo(x):
        hi = x.astype(f8)
        lo = (x - hi.astype(np.float64)).astype(f8)
        return np.stack([hi, lo], axis=1)

    def pack_w(w_slice):
        # [O, H] -> lhsT [H, O] -> [128, 2(hl), 8(tp), 2(ti), O] fp8 x64
        wT = w_slice.T.astype(np.float64) * WS
        O = wT.shape[1]
        base = wT.reshape(NHT // 2, 2, P, O).transpose(2, 0, 1, 3)
        return np.ascontiguousarray(hi_lo(base))

    hs_packed = []
    for b in range(B):
        hsT = hidden_states[b].T.astype(np.float64)        # [H, S]
        # [p, sc, 2(hl), 8(tp), 2(ti), 4(q), s128] fp8 hi/lo
        hs6 = hsT.reshape(NHT // 2, 2, P, NCH, 4, P).transpose(2, 3, 0, 1, 4, 5)
        hi = hs6.astype(f8)
        lo = (hs6 - hi.astype(np.float64)).astype(f8)
        hs_packed.append(np.ascontiguousarray(np.stack([hi, lo], axis=2)))

    in_maps = []
    for core in range(8):
        b, g = divmod(core, 4)
        woT = wo[:, 512 * g:512 * (g + 1)].T.astype(np.float64)  # [512, H]
        wo_r = np.ascontiguousarray(
            woT.reshape(NQC, P, H).transpose(1, 0, 2)).astype(bf)
        in_maps.append({
            "hs": hs_packed[b],
            "wq": pack_w(wq[512 * g:512 * (g + 1), :]),
            "wk": pack_w(wk[256 * g:256 * (g + 1), :]),
            "wv": pack_w(wv[256 * g:256 * (g + 1), :]),
            "wo": wo_r,
            "tabs": tabs, "msk": msk,
        })
    return in_maps


def _postprocess(results):
    out = np.empty((B, S, H), np.float32)
    for b in range(B):
        acc = np.zeros((H, S), np.float32)
        for g in range(4):
            y_r = results[4 * b + g]["y"].astype(np.float32)  # [128, 16, S]
            acc += y_r.transpose(1, 0, 2).reshape(H, S)
        out[b] = acc.T
    return out


def kernel(hidden_states, wq, wk, wv, wo, q_norm_weight, k_norm_weight):
    nc = _build_nc()
    in_maps = _host_inputs(hidden_states, wq, wk, wv, wo,
                           q_norm_weight, k_norm_weight)
    res = run_bass_kernel_spmd(nc, in_maps, list(range(8)))
    return _postprocess(res.results)


# revision 81
# speedup vs baseline: 1.0306x; 1.0010x over previous
"""Gemma3 sliding-window attention on 8 Trainium2 NeuronCores.

Sharding: core c handles batch b=c//4 and head-group g=c%4 (4 of 16 q heads,
2 of 8 kv heads). wq/wk/wv column-split, wo row-split; the 4 partial outputs
per batch are summed on host (no device collectives).

v2: all matmul operands in bf16 (fp32 PSUM accumulation), single-instruction
batched DMA loads from host-prepacked layouts, per-q-tile(128) attention with
the softmax denominator computed as a 129th V column in a [q,d]-oriented PV
matmul, XBAR DMA transposes to return attn to [d,q] for the output
projection, and software-pipelined instruction issue so the PE never waits
on the RMSNorm/RoPE vector chains.
"""

import math
import numpy as np
import ml_dtypes

import concourse.bacc as bacc
import concourse.mybir as mybir
import concourse.tile as tile
from concourse.bass_utils import run_bass_kernel_spmd

dt = mybir.dt
AFT = mybir.ActivationFunctionType
BF = dt.bfloat16
F32 = dt.float32

B, S, H = 2, 2048, 2048
NQC, NKVC, D = 4, 2, 128          # per-core heads
WIN = 1024
EPS = 1e-6
THETA = 10000.0
P = 128
SCP = 512                          # phase-1 seq chunk
NCH = S // SCP                     # 4
NHT = H // P                       # 16
NST = S // P                       # 16
WT = WIN // P                      # 8 (window in tiles)
LAG = 2                            # attention PV pipeline depth (pair units)

_CACHE = {}


def _groups_for(t0, u0):
    """k-tile groups for one q tile: runs of <=4 tiles, diagonal tile alone
    last (so its [128,128] exp/mask stays separate)."""
    ts = list(range(t0, u0 + 1))
    if len(ts) == 1:
        return [ts]
    body, diag = ts[:-1], ts[-1:]
    gs = [body[i:i + 4] for i in range(0, len(body), 4)]
    gs.append(diag)
    return gs


def _build_nc():
    if "nc" in _CACHE:
        return _CACHE["nc"]
    nc = bacc.Bacc("TRN2", target_bir_lowering=False, debug=False, num_devices=8)

    F8 = dt.float8e4
    DR = mybir.MatmulPerfMode.DoubleRow
    # hi/lo fp8 pairs: x ~= hi + lo to ~0.1% rms; DoubleRow matmuls run the
    # (hi,hi), (hi,lo), (lo,hi) cross terms at 0.5 cyc/row over ht-pairs.
    hs_d = nc.dram_tensor("hs", [P, NCH, 2, NHT // 2, 2, 4, P], F8,
                          kind="ExternalInput").ap()
    wq_d = nc.dram_tensor("wq", [P, 2, NHT // 2, 2, NQC * D], F8,
                          kind="ExternalInput").ap()
    wk_d = nc.dram_tensor("wk", [P, 2, NHT // 2, 2, NKVC * D], F8,
                          kind="ExternalInput").ap()
    wv_d = nc.dram_tensor("wv", [P, 2, NHT // 2, 2, NKVC * D], F8,
                          kind="ExternalInput").ap()
    wo_d = nc.dram_tensor("wo", [P, NQC, H], BF, kind="ExternalInput").ap()
    tabs_d = nc.dram_tensor("tabs", [P, 4, S], BF, kind="ExternalInput").ap()
    msk_d = nc.dram_tensor("msk", [P, 2, 2, P], BF, kind="ExternalInput").ap()
    y_d = nc.dram_tensor("y", [P, NHT, S], BF, kind="ExternalOutput").ap()

    with nc.allow_low_precision(reason="bf16 kernel; rel-err budget 2e-2"), \
         tile.TileContext(nc) as tc:
        with (
            tc.tile_pool(name="const", bufs=1) as cpool,
            tc.tile_pool(name="qkv", bufs=1) as qkv,
            tc.tile_pool(name="wts", bufs=1) as wts,
        ):
            msk_sb = cpool.tile([P, 2, 2, P], BF, tag="msk")
            ones_sb = cpool.tile([P, P], BF, tag="ones")
            eps_sb = cpool.tile([P, 1], F32, tag="eps")
            nc.vector.memset(ones_sb[:], 1.0)
            nc.vector.memset(eps_sb[:], EPS)
            dm_sb = msk_sb[:, 0, :, :]
            em_sb = msk_sb[:, 1, :, :]

            # weight loads: wv first (v-projection is the startup filler work),
            # then wk (k heads run before q heads), wq, wo last-needed.
            wv_sb = wts.tile([P, 2, NHT // 2, 2, NKVC * D], F8, tag="wv")
            wk_sb = wts.tile([P, 2, NHT // 2, 2, NKVC * D], F8, tag="wk")
            wq_sb = wts.tile([P, 2, NHT // 2, 2, NQC * D], F8, tag="wq")
            wo_sb = wts.tile([P, NQC, H], BF, tag="wo")

            qn_sb = qkv.tile([P, NQC, S], BF, tag="qn")
            kn_sb = qkv.tile([P, NKVC, S], BF, tag="kn")
            v_sb = qkv.tile([P, NST, NKVC, D + 1], BF, tag="v")
            nc.vector.memset(v_sb[:, :, :, D:D + 1], 64.0)

            # ---------------- phase 1: QKV projections + RMSNorm + RoPE ----
            # per (chunk, head): PE proj chain -> Act copy -> DVE square /
            # rope muls; the sum-of-squares and rotation matmuls for head m
            # are issued after head m+1's projection chain so PE never waits.
            with (
                tc.tile_pool(name="hsp", bufs=3) as hsp,
                tc.tile_pool(name="tabp", bufs=2) as tabp,
                tc.tile_pool(name="cpp", bufs=5) as cpp,
                tc.tile_pool(name="t1", bufs=3) as t1p,
                tc.tile_pool(name="t2", bufs=3) as t2p,
                tc.tile_pool(name="t3", bufs=3) as t3p,
                tc.tile_pool(name="t4", bufs=3) as t4p,
                tc.tile_pool(name="t5", bufs=4) as t5p,
                tc.tile_pool(name="t6", bufs=4) as t6p,
                tc.tile_pool(name="pp", bufs=3, space="PSUM") as ppp,
                tc.tile_pool(name="pvb", bufs=2, space="PSUM") as pvbp,
                tc.tile_pool(name="psv", bufs=3, space="PSUM") as psvp,
            ):
                # heads order: k0, k1, q0..q3 (wk arrives before wq)
                HEADS = [("k", 0), ("k", 1), ("q", 0), ("q", 1), ("q", 2), ("q", 3)]
                pend = []  # deferred norm/rope finishes (2-deep pipeline)

                def proj_chain(out_ps, w_sb8, mcols, hs_t):
                    first = True
                    for wi, xi in ((0, 0), (0, 1), (1, 0)):
                        for tp in range(NHT // 2):
                            nc.tensor.matmul(
                                out_ps[:], w_sb8[:, wi, tp, :, mcols],
                                hs_t[:, xi, tp, :, :, :],
                                perf_mode=DR, start=first,
                                stop=(wi == 1 and tp == NHT // 2 - 1))
                            first = False

                def v_chain(out_ps, hs_t, ss):
                    first = True
                    for wi, xi in ((0, 0), (0, 1), (1, 0)):
                        for tp in range(NHT // 2):
                            nc.tensor.matmul(
                                out_ps[:], hs_t[:, xi, tp, :, ss, :],
                                wv_sb[:, wi, tp, :, :],
                                perf_mode=DR, start=first,
                                stop=(wi == 1 and tp == NHT // 2 - 1))
                            first = False

                def finish(st):
                    kind, m, pp, cp, u_t, s0, tab_t, rot = st
                    sq = t1p.tile([P, SCP], BF, tag="sq")
                    nc.scalar.square(sq[:], cp[:])
                    vb = pvbp.tile([P, SCP], F32, tag="vb")
                    nc.tensor.matmul(vb[:], ones_sb[:], sq[:], start=True, stop=True)
                    sd = t2p.tile([P, SCP], F32, tag="sd")
                    nc.scalar.activation(sd[:], vb[:], AFT.Sqrt, bias=eps_sb[:],
                                         scale=1.0 / D)
                    inv = t3p.tile([P, SCP], BF, tag="inv")
                    nc.vector.reciprocal(inv[:], sd[:])
                    tsin = t4p.tile([P, SCP], BF, tag="tsin")
                    sin_t = tab_t[:, 1 if kind == "q" else 3, :]
                    nc.vector.tensor_mul(tsin[:], rot[:], sin_t)
                    nc.vector.tensor_add(u_t[:], u_t[:], tsin[:])
                    dst = qn_sb if kind == "q" else kn_sb
                    nc.vector.tensor_mul(dst[:, m, s0:s0 + SCP], u_t[:], inv[:])

                chunk_tiles = {}

                def load_chunk(sc):
                    s0 = sc * SCP
                    hs_sb = hsp.tile([P, 2, NHT // 2, 2, 4, P], F8, tag="hs",
                                     name="hs_sb")
                    nc.sync.dma_start(out=hs_sb[:], in_=hs_d[:, sc])
                    tab_sb = tabp.tile([P, 4, SCP], BF, tag="tab",
                                       name="tab_sb")
                    nc.sync.dma_start(out=tab_sb[:],
                                      in_=tabs_d[:, :, s0:s0 + SCP])
                    chunk_tiles[sc] = (hs_sb, tab_sb)

                for sc in range(NCH):
                    s0 = sc * SCP
                    if sc > 0:
                        hs_sb, tab_sb = chunk_tiles[sc]
                    else:
                        hs_sb = hsp.tile([P, 2, NHT // 2, 2, 4, P], F8,
                                         tag="hs", name="hs_sb")
                    if sc == 0:
                        # startup-critical order: hi parts first (the hi-hi
                        # chain leads each accumulation), v before k/q.
                        nc.sync.dma_start(out=wv_sb[:, 0], in_=wv_d[:, 0])
                        nc.sync.dma_start(out=hs_sb[:, 0, 0:4], in_=hs_d[:, 0, 0, 0:4])
                        nc.sync.dma_start(out=hs_sb[:, 0, 4:8], in_=hs_d[:, 0, 0, 4:8])
                        nc.sync.dma_start(out=wv_sb[:, 1], in_=wv_d[:, 1])
                        nc.sync.dma_start(out=hs_sb[:, 1], in_=hs_d[:, 0, 1])
                        nc.sync.dma_start(out=wk_sb[:], in_=wk_d[:])
                        nc.sync.dma_start(out=msk_sb[:], in_=msk_d[:])
                        nc.sync.dma_start(out=wq_sb[:], in_=wq_d[:])
                    if sc == 0:
                        tab_sb = tabp.tile([P, 4, SCP], BF, tag="tab",
                                           name="tab_sb")
                        nc.sync.dma_start(out=tab_sb[:],
                                          in_=tabs_d[:, :, s0:s0 + SCP])
                        nc.sync.dma_start(out=wo_sb[:], in_=wo_d[:])

                    # v projection: natural [seq, d] layout + filler work
                    for ss in range(SCP // P):
                        st_g = sc * (SCP // P) + ss
                        pv = psvp.tile([P, NKVC * D], F32, tag="pv")
                        v_chain(pv, hs_sb, ss)
                        nc.vector.tensor_copy(v_sb[:, st_g, :, 0:D], pv[:])
                        if ss == 0 and pend:
                            # finish the previous chunk's last head here: its
                            # DVE ops land ahead of the remaining v copies
                            finish(pend.pop(0))
                        if ss == 0 and sc + 1 < NCH:
                            load_chunk(sc + 1)  # prefetch ahead of rot DMAs

                    for kind, m in HEADS:
                        w_sb = wq_sb if kind == "q" else wk_sb
                        pp = ppp.tile([P, SCP], F32, tag="pp")
                        proj_chain(pp, w_sb, slice(m * D, (m + 1) * D), hs_sb)
                        cp = cpp.tile([P, SCP], BF, tag="cp")
                        nc.scalar.copy(cp[:], pp[:])
                        # rotate_half as a partition-shift DMA (sign and norm
                        # weight are folded into the sin tables on the host)
                        rot = t6p.tile([P, SCP], BF, tag="rot")
                        nc.sync.dma_start(out=rot[0:64, :], in_=cp[64:128, :])
                        nc.sync.dma_start(out=rot[64:128, :], in_=cp[0:64, :])
                        u_t = t5p.tile([P, SCP], BF, tag="u")
                        cos_t = tab_sb[:, 0 if kind == "q" else 2, :]
                        nc.vector.tensor_mul(u_t[:], cp[:], cos_t)
                        pend.append((kind, m, pp, cp, u_t, s0, tab_sb, rot))
                        if len(pend) > 1:
                            finish(pend.pop(0))
                for st in pend:
                    finish(st)

            # ---------------- phase 2+3: attention + output projection -----
            with (
                tc.tile_pool(name="pb", bufs=LAG + 2) as pbp,
                tc.tile_pool(name="invp", bufs=4) as invp,
                tc.tile_pool(name="aq", bufs=2) as aqp,
                tc.tile_pool(name="aT", bufs=2) as aTp,
                tc.tile_pool(name="ysb", bufs=2) as ysp,
                tc.tile_pool(name="psc", bufs=4, space="PSUM") as pscp,
                tc.tile_pool(name="pa", bufs=2, space="PSUM") as pap,
                tc.tile_pool(name="psy", bufs=2, space="PSUM") as psyp,
            ):
                queue = []
                slab_tiles = {}

                def emit_scores(kvh, u0):
                    # paired unit: both q heads of this kv head at once
                    h0 = 2 * kvh
                    t0 = max(0, u0 - WT)
                    n = u0 - t0 + 1
                    p_t = pbp.tile([P, WT + 1, 2, P], BF, tag="p", name="p_t")
                    qn_sl = qn_sb[:, h0:h0 + 2, u0 * P:(u0 + 1) * P]
                    for g0 in range(0, n, 2):
                        gn = min(2, n - g0)
                        sc_t = pscp.tile([P, 2, 2, P], F32, tag="sc",
                                         name="sc_t")
                        for i in range(gn):
                            t = t0 + g0 + i
                            nc.tensor.matmul(
                                sc_t[:, i, :, :],
                                kn_sb[:, kvh, t * P:(t + 1) * P],
                                qn_sl, start=True, stop=True)
                        nc.scalar.activation(p_t[:, g0:g0 + gn, :, :],
                                             sc_t[:, 0:gn, :, :], AFT.Exp)
                        for i in range(gn):
                            t = t0 + g0 + i
                            if t == u0:
                                blk = p_t[:, g0 + i, :, :]
                                nc.vector.tensor_mul(blk, blk, dm_sb)
                            elif u0 >= WT and t == u0 - WT:
                                blk = p_t[:, g0 + i, :, :]
                                nc.vector.tensor_mul(blk, blk, em_sb)
                    return (kvh, u0, t0, p_t)

                def emit_pv(st):
                    kvh, u0, t0, p_t = st
                    n = u0 - t0 + 1
                    slab = slab_tiles[u0 // 4]
                    for hh in range(2):
                        h = 2 * kvh + hh
                        a_t = pap.tile([P, D + 1], F32, tag="a")
                        for i in range(n):
                            nc.tensor.matmul(
                                a_t[:], p_t[:, i, hh, :],
                                v_sb[:, t0 + i, kvh, :],
                                start=(i == 0), stop=(i == n - 1))
                        inv = invp.tile([P, 1], F32, tag="inv")
                        nc.vector.reciprocal(inv[:], a_t[:, D:D + 1])
                        nc.vector.tensor_scalar_mul(slab[:, h, u0 % 4, :],
                                                    a_t[:, 0:D], inv[:])

                def emit_transpose(s4, h):
                    # Act HWDGE queue: avoids head-of-line blocking behind the
                    # SP queue's y-out DMAs (which wait on DVE copies). Issued
                    # one per unit so the Act SEQ time (~0.7us per DMA) does
                    # not delay exp dispatch in a lump.
                    if h == 0:
                        aT = aTp.tile([P, NQC, 4, P], BF, tag="aT", name="aT")
                        slab_tiles[("T", s4)] = aT
                    aT = slab_tiles[("T", s4)]
                    slab = slab_tiles[s4]
                    nc.sync.dma_start_transpose(out=aT[:, h, :, :],
                                                in_=slab[:, h, :, :])

                op_queue = []  # (s4, mo) outproj chains, spread across units
                op_state = {}

                def emit_outproj_chain():
                    if not op_queue:
                        return
                    s4, mo = op_queue.pop(0)
                    aT = slab_tiles[("T", s4)]
                    mog, mo4 = divmod(mo, 4)
                    if mo4 == 0:
                        op_state["y"] = ysp.tile([P, 4, SCP], BF, tag="y",
                                                 name="y_t")
                    y_t = op_state["y"]
                    yp = psyp.tile([P, SCP], F32, tag="yp")
                    for h in range(NQC):
                        nc.tensor.matmul(
                            yp[:], wo_sb[:, h, mo * P:(mo + 1) * P],
                            aT[:, h, :, :],
                            start=(h == 0), stop=(h == NQC - 1))
                    nc.vector.tensor_copy(y_t[:, mo4, :], yp[:])
                    if mo4 == 3:
                        nc.sync.dma_start(
                            out=y_d[:, mog * 4:(mog + 1) * 4,
                                    s4 * SCP:(s4 + 1) * SCP],
                            in_=y_t[:])

                def emit_outproj(s4):
                    op_queue.extend((s4, mo) for mo in range(NHT))

                # descending u0: the big steady-state units come first and
                # prime the PV pipeline; the small ramp units land at the end
                # where the outproj slabs provide PE filler work.
                for u0 in range(NST - 1, -1, -1):
                    if u0 % 4 == 3:
                        slab_tiles[u0 // 4] = aqp.tile([P, NQC, 4, P], BF,
                                                       tag="aq", name="aq")
                    for kvh in range(NKVC):
                        queue.append(emit_scores(kvh, u0))
                        if len(queue) > LAG:
                            emit_pv(queue.pop(0))
                        if u0 == 0 and queue:
                            emit_pv(queue.pop(0))  # drain early for the tail
                        if u0 % 4 == 2 and u0 <= NST - 6:
                            emit_transpose(u0 // 4 + 1, 2 * kvh)
                            emit_transpose(u0 // 4 + 1, 2 * kvh + 1)
                        emit_outproj_chain()
                        if u0 > 1:
                            emit_outproj_chain()  # hold chains back for the
                            # tail drain (fills the final transpose latency)
                    if u0 % 4 == 2 and u0 <= NST - 6:
                        emit_outproj(u0 // 4 + 1)
                while queue:
                    emit_pv(queue.pop(0))
                    emit_outproj_chain()
                    emit_outproj_chain()
                # first slab (last processed): split transposes across both
                # HWDGE queues to halve the serial latency in the tail
                aT = aTp.tile([P, NQC, 4, P], BF, tag="aT", name="aT")
                slab_tiles[("T", 0)] = aT
                slab = slab_tiles[0]
                for h in range(NQC):
                    eng = nc.scalar if h % 2 == 0 else nc.sync
                    eng.dma_start_transpose(out=aT[:, h, :, :],
                                            in_=slab[:, h, :, :])
                emit_outproj(0)
                while op_queue:
                    emit_outproj_chain()

    nc.compile()
    _CACHE["nc"] = nc
    return nc


def _host_inputs(hidden_states, wq, wk, wv, wo, q_norm_weight, k_norm_weight):
    """Per-core input dicts (8 cores: c = 4*b + g)."""
    bf = ml_dtypes.bfloat16
    f = np.float32
    scale = 1.0 / math.sqrt(D)
    inv_freq = 1.0 / (THETA ** (np.arange(0, D, 2, dtype=np.float64) / D))
    t = np.arange(S, dtype=np.float64)
    freqs = np.outer(t, inv_freq)
    emb = np.concatenate([freqs, freqs], axis=-1)          # [S, D]
    cosT = np.cos(emb).T.astype(np.float64)                # [D, S]
    sinT = np.sin(emb).T.astype(np.float64)
    qw = (1.0 + q_norm_weight).astype(np.float64)
    kw = (1.0 + k_norm_weight).astype(np.float64)

    # rotate_half is done on-device as a plain partition-shift DMA
    # (rot[d] = x[(d+64)%128]); the rotation sign and the SHIFTED norm
    # weight are folded into the sin tables here:
    #   sin_eff[d] = sgn(d) * sin[d] * w[(d+64)%128],  sgn = -1 for d<64
    hh = D // 2
    sgn = np.where(np.arange(D) < hh, -1.0, 1.0)
    qw_sh = np.roll(qw, -hh)   # w[(d+64)%128]
    kw_sh = np.roll(kw, -hh)
    tabs = np.stack([
        cosT * qw[:, None] * scale,
        sinT * (sgn * qw_sh)[:, None] * scale,
        cosT * kw[:, None],
        sinT * (sgn * kw_sh)[:, None],
    ], axis=1).astype(bf)                                  # [D, 4, S]

    r = np.arange(P)[:, None]
    c = np.arange(P)[None, :]
    dmask = np.where(c >= r, 1.0, 0.0)                     # [k, q]: q >= k
    emask = np.where(c < r, 1.0, 0.0)                      # [k, q]: q < k
    msk = np.stack([dmask, dmask, emask, emask],
                   axis=1).reshape(P, 2, 2, P).astype(bf)  # [k, dm/em, hdup, q]

    f8 = ml_dtypes.float8_e4m3
    WS = 64.0  # weight scale: fp8 sweet spot; absorbed by RMSNorm (q/k) and
               # by the 64-valued ones-column of v_ext (v)

    def hi_lo(x):
        hi = x.astype(f8)
        lo = (x - hi.astype(np.float64)).astype(f8)
        return np.stack([hi, lo], axis=1)

    def pack_w(w_slice):
        # [O, H] -> lhsT [H, O] -> [128, 2(hl), 8(tp), 2(ti), O] fp8 x64
        wT = w_slice.T.astype(np.float64) * WS
        O = wT.shape[1]
        base = wT.reshape(NHT // 2, 2, P, O).transpose(2, 0, 1, 3)
        return np.ascontiguousarray(hi_lo(base))

    hs_packed = []
    for b in range(B):
        hsT = hidden_states[b].T.astype(np.float64)        # [H, S]
        # [p, sc, 2(hl), 8(tp), 2(ti), 4(q), s128] fp8 hi/lo
        hs6 = hsT.reshape(NHT // 2, 2, P, NCH, 4, P).transpose(2, 3, 0, 1, 4, 5)
        hi = hs6.astype(f8)
        lo = (hs6 - hi.astype(np.float64)).astype(f8)
        hs_packed.append(np.ascontiguousarray(np.stack([hi, lo], axis=2)))

    in_maps = []
    for core in range(8):
        b, g = divmod(core, 4)
        woT = wo[:, 512 * g:512 * (g + 1)].T.astype(np.float64)  # [512, H]
        wo_r = np.ascontiguousarray(
            woT.reshape(NQC, P, H).transpose(1, 0, 2)).astype(bf)
        in_maps.append({
            "hs": hs_packed[b],
            "wq": pack_w(wq[512 * g:512 * (g + 1), :]),
            "wk": pack_w(wk[256 * g:256 * (g + 1), :]),
            "wv": pack_w(wv[256 * g:256 * (g + 1), :]),
            "wo": wo_r,
            "tabs": tabs, "msk": msk,
        })
    return in_maps


def _postprocess(results):
    out = np.empty((B, S, H), np.float32)
    for b in range(B):
        acc = np.zeros((H, S), np.float32)
        for g in range(4):
            y_r = results[4 * b + g]["y"].astype(np.float32)  # [128, 16, S]
            acc += y_r.transpose(1, 0, 2).reshape(H, S)
        out[b] = acc.T
    return out


def kernel(hidden_states, wq, wk, wv, wo, q_norm_weight, k_norm_weight):
    nc = _build_nc()
    in_maps = _host_inputs(hidden_states, wq, wk, wv, wo,
                           q_norm_weight, k_norm_weight)
    res = run_bass_kernel_spmd(nc, in_maps, list(range(8)))
    return _postprocess(res.results)
